# revision 8
# baseline (speedup 1.0000x reference)
"""LSTM (B=131072, T=10, INP=HID=64) + linear head, data-parallel on 8 TRN2 cores.

v5 layout (per core, B_loc=16384, 16 "units" of two 512-col groups A/B):
  - Feature-major: features on SBUF partitions, batch on the free dim. PSUM
    per unit-step: [128, 4, NB] banks (i, f, o, g), bank = [gate_A(0:64);
    gate_B(64:128)], so all elementwise ops run 128 lanes wide. A-group rhs is
    [h; x] on 128 partitions, B-group rhs is [x; h] (so each group's h-write
    stays partition-aligned); per-group weight copies match (all bf16).
  - Bias: banks 0-1 seeded by K=1 matmuls on PE, banks 2-3 by a single DVE
    copy from an SBUF bias image; gate matmuls accumulate on top. One merged
    sigmoid covers all 4 banks (g weights/bias pre-doubled so
    tanh(g) = 2*sig(2g)-1); tanh(c) batched across unit pairs.
  - Elementwise: i*g and f*c products on Pool(GPSIMD); Gt fix, c-add, h-muls,
    head staging on DVE.
  - x arrives in multi-step DMA epochs, staggered across units so the SP DMA
    queue never bursts at a shared epoch boundary.
"""

import numpy as np
import ml_dtypes

import concourse.bass as bass
import concourse.mybir as mybir
from concourse import bacc
import concourse.tile as tile

HID = 64
INP = 64
T = 10
B = 131072
NCORES = 8
B_LOC = B // NCORES   # 16384
NB = 512              # batch columns per group
NUNITS = B_LOC // (2 * NB)  # 16 units of (A, B) groups
EPOCH = 3             # max steps of x per DMA

BF = mybir.dt.bfloat16
F32 = mybir.dt.float32
AF = mybir.ActivationFunctionType
ALU = mybir.AluOpType

# psum gate-slice order: 0=i, 1=f, 2=o, 3=g ; torch block order i,f,g,o
SLICE_TO_TORCH_GATE = [0, 1, 3, 2]


def _epoch_plan(u):
    """Staggered epoch starts for unit u: list of (start, len)."""
    first = 1 + (u % EPOCH)
    starts, t0 = [], 0
    n = first
    while t0 < T:
        n = min(n, T - t0)
        starts.append((t0, n))
        t0 += n
        n = EPOCH
    return starts


EP_PLANS = [_epoch_plan(u) for u in range(NUNITS)]
# map (u, t) -> (epoch_index, slot)
EP_AT = []
for u in range(NUNITS):
    m = {}
    for ei, (t0, n) in enumerate(EP_PLANS[u]):
        for k in range(n):
            m[t0 + k] = (ei, k)
    EP_AT.append(m)
N_EP_MAX = max(len(p) for p in EP_PLANS)


def emit_lstm(tc, aps):
    nc = tc.nc
    xa, xb, WAd, WBd, BWd, BId, WOd, BOd, y = (
        aps["xa"], aps["xb"], aps["WAd"], aps["WBd"], aps["BWd"], aps["BId"],
        aps["WOd"], aps["BOd"], aps["y"])

    with (
        tc.tile_pool(name="const", bufs=1) as cpool,
        tc.tile_pool(name="rhs", bufs=1) as rpool,
        tc.tile_pool(name="cstate", bufs=2) as spool,
        tc.tile_pool(name="work", bufs=3) as wpool,
        tc.tile_pool(name="small", bufs=4) as qpool,
        tc.tile_pool(name="psum", bufs=2, space="PSUM") as ppool,
    ):
        # A-weights [k(128: h, x), slice, m] / B-weights [k(128: x, h), ...]
        WA = cpool.tile([128, 4, 64], BF)
        nc.sync.dma_start(out=WA, in_=WAd)
        WB = cpool.tile([128, 4, 64], BF)
        nc.sync.dma_start(out=WB, in_=WBd)
        BW = cpool.tile([1, 4, 128], BF)
        nc.sync.dma_start(out=BW, in_=BWd)
        BI = cpool.tile([128, 2, NB], BF)   # bias image for banks 2-3 (o, g)
        nc.sync.dma_start(out=BI, in_=BId)
        WO = cpool.tile([64, 1], BF)
        nc.sync.dma_start(out=WO, in_=WOd)
        BO = cpool.tile([1, 1], BF)
        nc.sync.dma_start(out=BO, in_=BOd)
        ones_sb = cpool.tile([1, NB], BF)
        nc.vector.memset(ones_sb, 1.0)

        def alloc_rhs(u, ei, t):
            n = EP_PLANS[u][ei][1]
            ra = rpool.tile([128, EPOCH, NB], BF, tag=f"ra{u}",
                            name=f"ra_{t}_{u}")
            rb = rpool.tile([128, EPOCH, NB], BF, tag=f"rb{u}",
                            name=f"rb_{t}_{u}")
            nc.sync.dma_start(out=ra[64:128, 0:n, :], in_=xa[ei, :, u, 0:n, :])
            nc.sync.dma_start(out=rb[0:64, 0:n, :], in_=xb[ei, :, u, 0:n, :])
            return ra, rb

        R = [alloc_rhs(u, 0, -1) for u in range(NUNITS)]
        CP = [None] * (NUNITS // 2)   # c state per unit pair [128, 2, NB]
        GS_prev = None
        CP_pending = None

        for t in range(T):
            last = t == T - 1
            for u in range(NUNITS):
                p = u // 2
                es = EP_AT[u][t][1]
                ra, rb = R[u]
                ps = ppool.tile([128, 4, NB], F32, tag="g", name=f"ps_{t}_{u}")
                for s in range(2):
                    nc.tensor.matmul(ps[:, s], BW[:, s, :], ones_sb,
                                     start=True, stop=False,
                                     skip_group_check=True)
                nc.vector.tensor_copy(out=ps[:, 2:4, :], in_=BI)
                for s in range(4):
                    lst = s == 3
                    if t == 0:
                        # h == 0: contract x only (K=64)
                        nc.tensor.matmul(ps[0:64, s], WA[64:128, s, :],
                                         ra[64:128, 0, :], start=False,
                                         stop=False, skip_group_check=True)
                        nc.tensor.matmul(ps[64:128, s], WB[0:64, s, :],
                                         rb[0:64, 0, :], start=False,
                                         stop=lst, skip_group_check=True)
                    else:
                        nc.tensor.matmul(ps[0:64, s], WA[:, s, :],
                                         ra[:, es, :], start=False,
                                         stop=False, skip_group_check=True)
                        nc.tensor.matmul(ps[64:128, s], WB[:, s, :],
                                         rb[:, es, :], start=False,
                                         stop=lst, skip_group_check=True)

                GS = wpool.tile([128, 4, NB], BF, tag="GS", name=f"gs_{t}_{u}")
                nc.scalar.activation(GS, ps, AF.Sigmoid)
                Gt = qpool.tile([128, NB], BF, tag="Gt", name=f"gt_{t}_{u}")
                # tanh(g) = 2*sigmoid(2g) - 1  (g weights/bias pre-doubled)
                nc.vector.tensor_scalar(Gt, GS[:, 3], 2.0, -1.0, ALU.mult, ALU.add)

                if u % 2 == 0:
                    CPnew = spool.tile([128, 2, NB], BF, tag=f"C{p}",
                                       name=f"c_{t}_{p}")
                    CP_pending = CPnew
                else:
                    CPnew = CP_pending
                if t == 0:
                    # c0 = i*g straight from the Pool multiply
                    nc.gpsimd.tensor_mul(CPnew[:, u % 2, :], GS[:, 0], Gt)
                else:
                    uu = qpool.tile([128, NB], BF, tag="uu", name=f"uu_{t}_{u}")
                    ww = qpool.tile([128, NB], BF, tag="ww", name=f"ww_{t}_{u}")
                    nc.gpsimd.tensor_mul(uu, GS[:, 0], Gt)
                    nc.gpsimd.tensor_mul(ww, GS[:, 1], CP[p][:, u % 2, :])
                    nc.vector.tensor_add(CPnew[:, u % 2, :], uu, ww)

                if u % 2 == 0:
                    GS_prev = GS
                    continue

                CP[p] = CP_pending
                # pair complete: tanh + h for both units of the pair
                TT = wpool.tile([128, 2, NB], BF, tag="TT", name=f"tt_{t}_{u}")
                nc.scalar.activation(TT, CP[p], AF.Tanh)
                for v in (u - 1, u):
                    GSv = GS_prev if v == u - 1 else GS
                    if not last:
                        ein, esn = EP_AT[v][t + 1]
                        if esn == 0:
                            R[v] = alloc_rhs(v, ein, t)
                        rav, rbv = R[v]
                        nc.vector.tensor_mul(rav[0:64, esn, :],
                                             GSv[0:64, 2], TT[0:64, v % 2])
                        nc.vector.tensor_mul(rbv[64:128, esn, :],
                                             GSv[64:128, 2], TT[64:128, v % 2])
                    else:
                        H = wpool.tile([64, 2, NB], BF, tag="H", name=f"h_{v}")
                        nc.vector.tensor_mul(H[:, 0, :], GSv[0:64, 2],
                                             TT[0:64, v % 2])
                        nc.vector.tensor_mul(H[:, 1, :], GSv[64:128, 2],
                                             TT[64:128, v % 2])
                        for g in range(2):
                            op = ppool.tile([1, NB], F32, tag="g",
                                            name=f"op_{v}_{g}")
                            nc.tensor.matmul(op, BO, ones_sb,
                                             start=True, stop=False,
                                             skip_group_check=True)
                            nc.tensor.matmul(op, WO, H[:, g, :],
                                             start=False, stop=True,
                                             skip_group_check=True)
                            ob = qpool.tile([1, NB], F32, tag="ob",
                                            name=f"ob_{v}_{g}")
                            nc.vector.tensor_copy(out=ob, in_=op)
                            nc.sync.dma_start(out=y[v, g], in_=ob)


def prep_weights(W_ih, W_hh, b_ih, b_hh, W_out, b_out):
    """Host-side packing (numpy). Returns DRAM arrays for the kernel."""
    bf16 = ml_dtypes.bfloat16
    WA = np.zeros((128, 4, 64), np.float32)     # A rhs layout [h; x]
    WB = np.zeros((128, 4, 64), np.float32)     # B rhs layout [x; h]
    BW = np.zeros((1, 4, 128), np.float32)
    b = (b_ih + b_hh).astype(np.float32)
    for s, gi in enumerate(SLICE_TO_TORCH_GATE):
        blk_ih = W_ih[gi * 64:(gi + 1) * 64, :].astype(np.float32)
        blk_hh = W_hh[gi * 64:(gi + 1) * 64, :].astype(np.float32)
        scale = 2.0 if s == 3 else 1.0
        WA[0:64, s, :] = blk_hh.T * scale
        WA[64:128, s, :] = blk_ih.T * scale
        WB[0:64, s, :] = blk_ih.T * scale
        WB[64:128, s, :] = blk_hh.T * scale
        bb = b[gi * 64:(gi + 1) * 64] * scale
        BW[0, s, 0:64] = bb
        BW[0, s, 64:128] = bb
    # bias image for banks 2 (o) and 3 (g)
    BI = np.stack([np.broadcast_to(BW[0, 2, :, None], (128, NB)),
                   np.broadcast_to(BW[0, 3, :, None], (128, NB))], axis=1)
    WO = W_out[0].astype(np.float32).reshape(64, 1)
    BO = np.full((1, 1), np.float32(b_out[0]))
    return {
        "WAd": WA.astype(bf16),
        "WBd": WB.astype(bf16),
        "BWd": BW.astype(bf16),
        "BId": np.ascontiguousarray(BI).astype(bf16),
        "WOd": WO.astype(bf16),
        "BOd": BO.astype(bf16),
    }


_BUILD_CACHE = {}


def build_nc():
    key = "nc_v5"
    if key in _BUILD_CACHE:
        return _BUILD_CACHE[key]
    nc = bacc.Bacc("TRN2", target_bir_lowering=False, debug=False)
    aps = {
        "xa": nc.dram_tensor("xa", [N_EP_MAX, INP, NUNITS, EPOCH, NB], BF,
                             kind="ExternalInput").ap(),
        "xb": nc.dram_tensor("xb", [N_EP_MAX, INP, NUNITS, EPOCH, NB], BF,
                             kind="ExternalInput").ap(),
        "WAd": nc.dram_tensor("WAd", [128, 4, 64], BF,
                              kind="ExternalInput").ap(),
        "WBd": nc.dram_tensor("WBd", [128, 4, 64], BF,
                              kind="ExternalInput").ap(),
        "BWd": nc.dram_tensor("BWd", [1, 4, 128], BF, kind="ExternalInput").ap(),
        "BId": nc.dram_tensor("BId", [128, 2, NB], BF,
                              kind="ExternalInput").ap(),
        "WOd": nc.dram_tensor("WOd", [64, 1], BF, kind="ExternalInput").ap(),
        "BOd": nc.dram_tensor("BOd", [1, 1], BF, kind="ExternalInput").ap(),
        "y": nc.dram_tensor("y", [NUNITS, 2, 1, NB], F32,
                            kind="ExternalOutput").ap(),
    }
    with tile.TileContext(nc) as tc:
        emit_lstm(tc, aps)
    nc.compile()
    _BUILD_CACHE[key] = nc
    return nc


def _pack_x(xc, grp):
    """[T, I, B_loc] -> [N_EP_MAX, I, NUNITS, EPOCH, NB] staggered epochs."""
    out = np.zeros((N_EP_MAX, INP, NUNITS, EPOCH, NB), np.float32)
    blk = xc.reshape(T, INP, NUNITS, 2, NB)[:, :, :, grp, :]  # [T, I, NU, NB]
    for u in range(NUNITS):
        for ei, (t0, n) in enumerate(EP_PLANS[u]):
            out[ei, :, u, 0:n, :] = blk[t0:t0 + n, :, u, :].transpose(1, 0, 2)
    return out


def make_in_maps(x, W_ih, W_hh, b_ih, b_hh, W_out, b_out):
    bf16 = ml_dtypes.bfloat16
    wd = prep_weights(W_ih, W_hh, b_ih, b_hh, W_out, b_out)
    xt = np.ascontiguousarray(x.transpose(1, 2, 0))   # [T, I, B] f32
    in_maps = []
    for c in range(NCORES):
        sl = xt[:, :, c * B_LOC:(c + 1) * B_LOC]
        xav = np.ascontiguousarray(_pack_x(sl, 0).astype(bf16))
        xbv = np.ascontiguousarray(_pack_x(sl, 1).astype(bf16))
        in_maps.append({"xa": xav, "xb": xbv, **wd})
    return in_maps


def kernel(x, W_ih, W_hh, b_ih, b_hh, W_out, b_out):
    from concourse.bass_utils import run_bass_kernel_spmd

    nc = build_nc()
    in_maps = make_in_maps(x, W_ih, W_hh, b_ih, b_hh, W_out, b_out)
    res = run_bass_kernel_spmd(nc, in_maps, core_ids=list(range(NCORES)))
    y = np.concatenate([res.results[c]["y"].reshape(B_LOC)
                        for c in range(NCORES)])
    return y.reshape(B, 1).astype(np.float32)


# revision 10
# speedup vs baseline: 1.0012x; 1.0012x over previous
"""LSTM (B=131072, T=10, INP=HID=64) + linear head, data-parallel on 8 TRN2 cores.

v6 layout (per core, B_loc=16384, 16 "units" of two 512-col groups A/B):
  - Feature-major: features on SBUF partitions, batch on the free dim. PSUM
    per unit-step: [128, 4, NB] banks (i, f, o, g), bank = [gate_A(0:64);
    gate_B(64:128)], so all elementwise ops run 128 lanes wide.
  - Both groups use rhs layout [h(0:64); x(64:128)] inside one persistent
    tile RAB[128, group, slot(4), NB] per unit, sharing a single weight copy.
    h_A lands aligned; h_B is written with a cross-partition output (legal:
    only tensor-op *inputs* must share a base partition). x is DMA'd two
    steps ahead into slot pairs, so DMA WAR waits are against long-retired
    readers and the SP queue never head-of-line blocks.
  - Bias: banks 0-1 seeded by K=1 matmuls on PE, banks 2-3 by one DVE copy
    from an SBUF bias image; gate matmuls accumulate on top. One merged
    sigmoid covers all 4 banks (g weights/bias pre-doubled so
    tanh(g) = 2*sig(2g)-1); tanh(c) batched across unit pairs.
  - Elementwise: i*g and f*c products on Pool(GPSIMD); Gt fix, c-add, h-muls,
    head staging on DVE.
"""

import numpy as np
import ml_dtypes

import concourse.bass as bass
import concourse.mybir as mybir
from concourse import bacc
import concourse.tile as tile

HID = 64
INP = 64
T = 10
B = 131072
NCORES = 8
B_LOC = B // NCORES   # 16384
NB = 512              # batch columns per group
NUNITS = B_LOC // (2 * NB)  # 16 units of (A, B) groups
NSLOT = 4             # rhs time slots (2-step DMA chunks, 2-step prefetch)

BF = mybir.dt.bfloat16
F32 = mybir.dt.float32
AF = mybir.ActivationFunctionType
ALU = mybir.AluOpType

# psum gate-slice order: 0=i, 1=f, 2=o, 3=g ; torch block order i,f,g,o
SLICE_TO_TORCH_GATE = [0, 1, 3, 2]


def emit_lstm(tc, aps):
    nc = tc.nc
    xab, Wd, BWd, BId, WOd, BOd, y = (
        aps["xab"], aps["Wd"], aps["BWd"], aps["BId"], aps["WOd"], aps["BOd"],
        aps["y"])

    with (
        tc.tile_pool(name="const", bufs=1) as cpool,
        tc.tile_pool(name="rhs", bufs=1) as rpool,
        tc.tile_pool(name="cstate", bufs=2) as spool,
        tc.tile_pool(name="work", bufs=3) as wpool,
        tc.tile_pool(name="small", bufs=3) as qpool,
        tc.tile_pool(name="psum", bufs=2, space="PSUM") as ppool,
    ):
        W = cpool.tile([128, 4, 64], BF)    # [k(h;x), slice, m]
        nc.sync.dma_start(out=W, in_=Wd)
        BW = cpool.tile([1, 4, 128], BF)
        nc.sync.dma_start(out=BW, in_=BWd)
        BI = cpool.tile([128, 2, NB], BF)   # bias image for banks 2-3 (o, g)
        nc.sync.dma_start(out=BI, in_=BId)
        WO = cpool.tile([64, 1], BF)
        nc.sync.dma_start(out=WO, in_=WOd)
        BO = cpool.tile([1, 1], BF)
        nc.sync.dma_start(out=BO, in_=BOd)
        ones_sb = cpool.tile([1, NB], BF)
        nc.vector.memset(ones_sb, 1.0)

        # persistent rhs tiles: [h(0:64); x(64:128)] x group x slot
        R = [rpool.tile([128, 2, NSLOT, NB], BF, tag=f"r{u}", name=f"r_{u}")
             for u in range(NUNITS)]

        def x_dma(u, chunk):
            sl = (2 * chunk) % NSLOT
            nc.sync.dma_start(out=R[u][64:128, :, sl:sl + 2, :],
                              in_=xab[chunk, :, u])

        for u in range(NUNITS):
            x_dma(u, 0)

        CP = [None] * (NUNITS // 2)   # c state per unit pair [128, 2, NB]
        GS_prev = None
        CP_pending = None

        for t in range(T):
            last = t == T - 1
            sl = t % NSLOT
            for u in range(NUNITS):
                p = u // 2
                ru = R[u]
                ps = ppool.tile([128, 4, NB], F32, tag="g", name=f"ps_{t}_{u}")
                for s in range(2):
                    nc.tensor.matmul(ps[:, s], BW[:, s, :], ones_sb,
                                     start=True, stop=False,
                                     skip_group_check=True)
                nc.vector.tensor_copy(out=ps[:, 2:4, :], in_=BI)
                for s in range(4):
                    lst = s == 3
                    if t == 0:
                        # h == 0: contract x only (K=64)
                        nc.tensor.matmul(ps[0:64, s], W[64:128, s, :],
                                         ru[64:128, 0, 0, :], start=False,
                                         stop=False, skip_group_check=True)
                        nc.tensor.matmul(ps[64:128, s], W[64:128, s, :],
                                         ru[64:128, 1, 0, :], start=False,
                                         stop=lst, skip_group_check=True)
                    else:
                        nc.tensor.matmul(ps[0:64, s], W[:, s, :],
                                         ru[:, 0, sl, :], start=False,
                                         stop=False, skip_group_check=True)
                        nc.tensor.matmul(ps[64:128, s], W[:, s, :],
                                         ru[:, 1, sl, :], start=False,
                                         stop=lst, skip_group_check=True)

                GS = wpool.tile([128, 4, NB], BF, tag="GS", name=f"gs_{t}_{u}")
                nc.scalar.activation(GS, ps, AF.Sigmoid)
                Gt = qpool.tile([128, NB], BF, tag="Gt", name=f"gt_{t}_{u}")
                # tanh(g) = 2*sigmoid(2g) - 1  (g weights/bias pre-doubled)
                nc.vector.tensor_scalar(Gt, GS[:, 3], 2.0, -1.0, ALU.mult, ALU.add)

                if u % 2 == 0:
                    CPnew = spool.tile([128, 2, NB], BF, tag=f"C{p}",
                                       name=f"c_{t}_{p}")
                    CP_pending = CPnew
                else:
                    CPnew = CP_pending
                if t == 0:
                    # c0 = i*g straight from the Pool multiply
                    nc.gpsimd.tensor_mul(CPnew[:, u % 2, :], GS[:, 0], Gt)
                else:
                    uu = qpool.tile([128, NB], BF, tag="uu", name=f"uu_{t}_{u}")
                    ww = qpool.tile([128, NB], BF, tag="ww", name=f"ww_{t}_{u}")
                    nc.gpsimd.tensor_mul(uu, GS[:, 0], Gt)
                    nc.gpsimd.tensor_mul(ww, GS[:, 1], CP[p][:, u % 2, :])
                    nc.vector.tensor_add(CPnew[:, u % 2, :], uu, ww)

                if u % 2 == 0:
                    GS_prev = GS
                    continue

                CP[p] = CP_pending
                # pair complete: tanh + h for both units of the pair
                TT = wpool.tile([128, 2, NB], BF, tag="TT", name=f"tt_{t}_{u}")
                nc.scalar.activation(TT, CP[p], AF.Tanh)
                for v in (u - 1, u):
                    GSv = GS_prev if v == u - 1 else GS
                    if not last:
                        if t % 2 == 0 and t + 2 < T:
                            x_dma(v, (t + 2) // 2)
                        sln = (t + 1) % NSLOT
                        rv = R[v]
                        nc.vector.tensor_mul(rv[0:64, 0, sln, :],
                                             GSv[0:64, 2], TT[0:64, v % 2])
                        nc.vector.tensor_mul(rv[0:64, 1, sln, :],
                                             GSv[64:128, 2], TT[64:128, v % 2])
                    else:
                        H = wpool.tile([64, 2, NB], BF, tag="H", name=f"h_{v}")
                        nc.vector.tensor_mul(H[:, 0, :], GSv[0:64, 2],
                                             TT[0:64, v % 2])
                        nc.vector.tensor_mul(H[:, 1, :], GSv[64:128, 2],
                                             TT[64:128, v % 2])
                        ob = qpool.tile([1, 2, NB], BF, tag="ob",
                                        name=f"ob_{v}")
                        for g in range(2):
                            op = ppool.tile([1, NB], F32, tag="g",
                                            name=f"op_{v}_{g}")
                            nc.tensor.matmul(op, BO, ones_sb,
                                             start=True, stop=False,
                                             skip_group_check=True)
                            nc.tensor.matmul(op, WO, H[:, g, :],
                                             start=False, stop=True,
                                             skip_group_check=True)
                            nc.vector.tensor_copy(out=ob[:, g, :], in_=op)
                        nc.sync.dma_start(out=y[v], in_=ob)


def prep_weights(W_ih, W_hh, b_ih, b_hh, W_out, b_out):
    """Host-side packing (numpy). Returns DRAM arrays for the kernel."""
    bf16 = ml_dtypes.bfloat16
    W = np.zeros((128, 4, 64), np.float32)      # rhs layout [h; x]
    BW = np.zeros((1, 4, 128), np.float32)
    b = (b_ih + b_hh).astype(np.float32)
    for s, gi in enumerate(SLICE_TO_TORCH_GATE):
        blk_ih = W_ih[gi * 64:(gi + 1) * 64, :].astype(np.float32)
        blk_hh = W_hh[gi * 64:(gi + 1) * 64, :].astype(np.float32)
        scale = 2.0 if s == 3 else 1.0
        W[0:64, s, :] = blk_hh.T * scale
        W[64:128, s, :] = blk_ih.T * scale
        bb = b[gi * 64:(gi + 1) * 64] * scale
        BW[0, s, 0:64] = bb
        BW[0, s, 64:128] = bb
    # bias image for banks 2 (o) and 3 (g)
    BI = np.stack([np.broadcast_to(BW[0, 2, :, None], (128, NB)),
                   np.broadcast_to(BW[0, 3, :, None], (128, NB))], axis=1)
    WO = W_out[0].astype(np.float32).reshape(64, 1)
    BO = np.full((1, 1), np.float32(b_out[0]))
    return {
        "Wd": W.astype(bf16),
        "BWd": BW.astype(bf16),
        "BId": np.ascontiguousarray(BI).astype(bf16),
        "WOd": WO.astype(bf16),
        "BOd": BO.astype(bf16),
    }


_BUILD_CACHE = {}


def build_nc():
    key = "nc_v6"
    if key in _BUILD_CACHE:
        return _BUILD_CACHE[key]
    nc = bacc.Bacc("TRN2", target_bir_lowering=False, debug=False)
    aps = {
        "xab": nc.dram_tensor("xab", [T // 2, INP, NUNITS, 2, 2, NB], BF,
                              kind="ExternalInput").ap(),
        "Wd": nc.dram_tensor("Wd", [128, 4, 64], BF,
                             kind="ExternalInput").ap(),
        "BWd": nc.dram_tensor("BWd", [1, 4, 128], BF, kind="ExternalInput").ap(),
        "BId": nc.dram_tensor("BId", [128, 2, NB], BF,
                              kind="ExternalInput").ap(),
        "WOd": nc.dram_tensor("WOd", [64, 1], BF, kind="ExternalInput").ap(),
        "BOd": nc.dram_tensor("BOd", [1, 1], BF, kind="ExternalInput").ap(),
        "y": nc.dram_tensor("y", [NUNITS, 1, 2, NB], BF,
                            kind="ExternalOutput").ap(),
    }
    with tile.TileContext(nc) as tc:
        emit_lstm(tc, aps)
    nc.compile()
    _BUILD_CACHE[key] = nc
    return nc


def make_in_maps(x, W_ih, W_hh, b_ih, b_hh, W_out, b_out):
    bf16 = ml_dtypes.bfloat16
    wd = prep_weights(W_ih, W_hh, b_ih, b_hh, W_out, b_out)
    xt = np.ascontiguousarray(x.transpose(1, 2, 0))   # [T, I, B] f32
    in_maps = []
    for c in range(NCORES):
        sl = xt[:, :, c * B_LOC:(c + 1) * B_LOC]
        # [T, I, B_loc] -> [T/2(chunk), 2(step), I, NU, 2(grp), NB]
        blk = sl.reshape(T // 2, 2, INP, NUNITS, 2, NB)
        xab = np.ascontiguousarray(
            blk.transpose(0, 2, 3, 4, 1, 5)).astype(bf16)
        in_maps.append({"xab": xab, **wd})
    return in_maps


def kernel(x, W_ih, W_hh, b_ih, b_hh, W_out, b_out):
    from concourse.bass_utils import run_bass_kernel_spmd

    nc = build_nc()
    in_maps = make_in_maps(x, W_ih, W_hh, b_ih, b_hh, W_out, b_out)
    res = run_bass_kernel_spmd(nc, in_maps, core_ids=list(range(NCORES)))
    y = np.concatenate([res.results[c]["y"].astype(np.float32).reshape(B_LOC)
                        for c in range(NCORES)])
    return y.reshape(B, 1).astype(np.float32)
